# revision 8
# baseline (speedup 1.0000x reference)
"""DenseDilatedKnnGraph kernel for 8x Trainium2 NeuronCores.

Architecture (v3, device retrieval + host rescore):
  Device (per core, 4096 query rows x 8192 candidates):
  - f32r (TF32) matmul computes quantized-embedded scores in PSUM:
      chunk matmul A (68 rows): 64 channel rows (tf32(2*a*K1) x tf32(b)),
      two bias rows ((OFFC - ||b||^2)*K1 split hi/lo), then a +BIG row and
      a -BIG row (BIG = 2^32). The in-chain +BIG add rounds the partial sum
      onto a 512-spaced grid, -BIG restores magnitude exactly, so PSUM holds
      q*512 with q = round((s + OFFC)*SC_q) in (0, 2^15).
      matmul B2 (1 row) accumulates the within-chunk column index (0..511):
      w = q*512 + idx, an exact integer < 2^24 in fp32.
  - per 512-wide group, one DVE max8 extracts the top-8 embedded values.
    No max_index / merge: the value carries its column; the group is the
    position in the output. 128 candidates per row ship to the host.
  Host: decodes candidates, rescores them exactly (fp64 dot products), and
  takes ranks 0,2,...,16 of the per-row sort. Quantization noise (~1.2e-4)
  only affects which candidates ship, with top-8-per-group slack; the final
  ordering is exact.

Sharding: 32768 query rows split across 8 cores (4096 rows = half a batch).
"""
import sys
import numpy as np

sys.path.insert(0, "/opt/trn_rl_repo")

import types
try:
    from antenv import axon_hooks  # noqa: F401
except Exception:
    import antenv
    _stub = types.ModuleType("antenv.axon_hooks")
    _stub.get_axon_ntff_profile_hook = lambda: None
    sys.modules["antenv.axon_hooks"] = _stub
    antenv.axon_hooks = _stub

from concourse import bass, tile, bacc  # noqa: E402
from concourse.bass_utils import run_bass_kernel_spmd  # noqa: E402

mybir = bass.mybir
dt = mybir.dt
AF = mybir.ActivationFunctionType

B, C, N = 4, 64, 8192
KOUT = 9
NCORES = 8
ROWS_PER_CORE = B * N // NCORES   # 4096
RT = ROWS_PER_CORE // 128         # 32 row-tiles per core
KROWS = C + 4                     # 64 ch + bias_h + bias_l + BIG + negBIG
CHUNK = 512                       # matmul free dim / PSUM bank / group width
NCHUNK = N // CHUNK               # 16
GRP = CHUNK
NGRP = N // GRP                   # 16
NCAND = NGRP * 8                  # 128 candidates per row

SC_q = 2**13 - 32                 # score quantum 1/8160
K1 = float(SC_q * GRP)            # lhsT channel scale
BIG = float(2.0**32)              # ulp(BIG) == GRP
OFFC = 3.012

_NC = None


def _tf32(x):
    """Round-to-nearest-even fp32 -> tf32 (10 explicit mantissa bits)."""
    x = np.ascontiguousarray(x, np.float32)
    u = x.view(np.uint32)
    keep = u & np.uint32(0xFFFFE000)
    rem = u & np.uint32(0x1FFF)
    half = np.uint32(0x1000)
    lsb = (u >> np.uint32(13)) & np.uint32(1)
    up = (rem > half) | ((rem == half) & (lsb == 1))
    return (keep + (up.astype(np.uint32) << np.uint32(13))).view(np.float32).copy()


def _build():
    nc = bacc.Bacc("TRN2", target_bir_lowering=False)
    packed_d = nc.declare_dram_parameter(
        "packed", [KROWS, ROWS_PER_CORE + N], dt.float32r, isOutput=False)
    idx_d = nc.declare_dram_parameter("idxrow", [1, 128 + N + 1024], dt.float32r, isOutput=False)
    idx128_d = nc.declare_dram_parameter("idx128", [128, CHUNK], dt.float32, isOutput=False)
    val_d = nc.declare_dram_parameter("val", [128, RT * NCAND], dt.float32, isOutput=True)

    with tile.TileContext(nc) as tc:
        with (
            tc.tile_pool(name="inp", bufs=1) as inp_pool,
            tc.tile_pool(name="srow", bufs=3) as srow_pool,
            tc.tile_pool(name="outp", bufs=1) as out_pool,
            tc.tile_pool(name="psum", bufs=6, space="PSUM") as psum_pool,
            tc.tile_pool(name="psumw", bufs=1, space="PSUM") as psumw_pool,
        ):
            packed = inp_pool.tile([KROWS, ROWS_PER_CORE + N], dt.float32r)
            aux = inp_pool.tile([1, 128 + N + 1024], dt.float32r)
            ones_l = aux[:, 0:128]
            idxrow = aux[:, 128:128 + N]
            plusbig = aux[:, 128 + N:128 + N + CHUNK]
            minusbig = aux[:, 128 + N + CHUNK:]
            idx128 = inp_pool.tile([128, CHUNK], dt.float32)
            tmp_pool_dummy = None
            # PE p-state warm-up (plain fp32) while the input DMA is in flight
            warm = inp_pool.tile([KROWS, 256], dt.float32)
            nc.gpsimd.memset(warm[:], 0.0)
            wps = psumw_pool.tile([128, 256], dt.float32)
            for _ in range(3):
                nc.tensor.matmul(wps[:], warm[:, 0:128], warm[:],
                                 start=True, stop=True)
            nc.sync.dma_start(out=aux[:], in_=idx_d[:])
            nc.sync.dma_start(out=idx128[:], in_=idx128_d[:])
            nc.sync.dma_start(out=packed[:, 0:128], in_=packed_d[:, 0:128])
            nc.sync.dma_start(out=packed[:, ROWS_PER_CORE:ROWS_PER_CORE + CHUNK],
                              in_=packed_d[:, ROWS_PER_CORE:ROWS_PER_CORE + CHUNK])
            nc.sync.dma_start(out=packed[:, ROWS_PER_CORE + CHUNK:ROWS_PER_CORE + 3 * CHUNK],
                              in_=packed_d[:, ROWS_PER_CORE + CHUNK:ROWS_PER_CORE + 3 * CHUNK])
            nc.sync.dma_start(out=packed[:, ROWS_PER_CORE + 3 * CHUNK:ROWS_PER_CORE + N // 2],
                              in_=packed_d[:, ROWS_PER_CORE + 3 * CHUNK:ROWS_PER_CORE + N // 2])
            nc.sync.dma_start(out=packed[:, ROWS_PER_CORE + N // 2:],
                              in_=packed_d[:, ROWS_PER_CORE + N // 2:])
            nc.sync.dma_start(out=packed[:, 128:ROWS_PER_CORE],
                              in_=packed_d[:, 128:ROWS_PER_CORE])
            lhsT_all = packed[:, 0:ROWS_PER_CORE]
            rhs_all = packed[:, ROWS_PER_CORE:]

            val_sb = out_pool.tile([128, RT * NCAND], dt.float32)

            for rt in range(RT):
                lhsT = lhsT_all[:, rt * 128:(rt + 1) * 128]
                s_sb = srow_pool.tile([128, N], dt.float32)
                vbase = rt * NCAND
                for ch in range(NCHUNK):
                    ps = psum_pool.tile([128, CHUNK], dt.float32)
                    nc.tensor.matmul(ps[:], lhsT[0:66, :],
                                     rhs_all[0:66, ch * CHUNK:(ch + 1) * CHUNK],
                                     start=True, stop=False)
                    # separate accumulate: PSUM-add of +2^32 rounds the full
                    # sum onto the 512 grid
                    if ch % 2 == 1 and ch != 15:
                        nc.tensor.matmul(ps[:], ones_l, plusbig[:],
                                         start=False, stop=False)
                        nc.tensor.matmul(ps[:], ones_l, minusbig[:],
                                         start=False, stop=False)
                        nc.tensor.matmul(ps[:], ones_l,
                                         idxrow[:, ch * CHUNK:(ch + 1) * CHUNK],
                                         start=False, stop=True)
                        nc.vector.max(val_sb[:, vbase + ch * 8:vbase + (ch + 1) * 8],
                                      ps[:])
                    else:
                        nc.tensor.matmul(ps[:], ones_l, plusbig[:],
                                         start=False, stop=True)
                        tmp = srow_pool.tile([128, CHUNK], dt.float32)
                        nc.scalar.activation(tmp[:], ps[:], AF.Copy, bias=-BIG)
                        dst = s_sb[:, ch * CHUNK:(ch + 1) * CHUNK]
                        nc.gpsimd.tensor_tensor(dst, tmp[:], idx128[:],
                                                mybir.AluOpType.add)
                        nc.vector.max(val_sb[:, vbase + ch * 8:vbase + (ch + 1) * 8],
                                      dst)

                if rt in (RT // 2 - 1, 3 * RT // 4 - 1, 7 * RT // 8 - 1):
                    lo = {RT // 2 - 1: 0, 3 * RT // 4 - 1: RT // 2,
                          7 * RT // 8 - 1: 3 * RT // 4}[rt]
                    hi = rt + 1
                    nc.sync.dma_start(out=val_d[:, lo * NCAND:hi * NCAND],
                                      in_=val_sb[:, lo * NCAND:hi * NCAND])

            q = 7 * RT // 8
            nc.sync.dma_start(out=val_d[:, q * NCAND:], in_=val_sb[:, q * NCAND:])

    nc.compile()
    return nc


def _decode_cands(results, c):
    """Per-core val -> [ROWS_PER_CORE, NCAND] global candidate column ids."""
    val = results[c]["val"].reshape(128, RT, NCAND).transpose(1, 0, 2) \
        .reshape(ROWS_PER_CORE, NCAND)
    w = val.astype(np.int64)
    idx = w & (GRP - 1)
    grp = np.arange(NCAND, dtype=np.int64)[None, :] >> 3
    return grp * GRP + idx


def _results_valid(results):
    """The best candidate of (almost) every row must be the row itself."""
    try:
        for c in range(NCORES):
            val = results[c]["val"]
            if not np.isfinite(val).all():
                return False
            w = val.reshape(128, RT, NCAND).astype(np.int64)
            if w.min() <= 0 or w.max() >= 2**24:
                return False
            wr = val.reshape(128, RT, NCAND).transpose(1, 0, 2).reshape(ROWS_PER_CORE, NCAND)
            k = np.argmax(wr, axis=1)
            wbest = wr[np.arange(ROWS_PER_CORE), k].astype(np.int64)
            glob0 = (k >> 3) * GRP + (wbest & (GRP - 1))
            r0 = (c % 2) * ROWS_PER_CORE
            expect = r0 + np.arange(ROWS_PER_CORE)
            if (glob0 == expect).mean() < 0.99:
                return False
    except Exception:
        return False
    return True


def _get_nc():
    global _NC
    if _NC is None:
        _NC = _build()
        try:
            zmaps = [{"packed": np.zeros((KROWS, ROWS_PER_CORE + N), np.float32),
                      "idxrow": np.zeros((1, 128 + N + 1024), np.float32),
                      "idx128": np.zeros((128, CHUNK), np.float32)}
                     for _ in range(NCORES)]
            run_bass_kernel_spmd(_NC, zmaps, list(range(NCORES)))
        except Exception:
            pass
    return _NC


def _normalize(x):
    x64 = np.asarray(x).astype(np.float64)              # (B,C,N,1)
    norm = np.sqrt((x64 * x64).sum(axis=1, keepdims=True))
    pts32 = (x64 / np.maximum(norm, 1e-12)).squeeze(-1).transpose(0, 2, 1).astype(np.float32)
    sq32 = (pts32.astype(np.float64) ** 2).sum(-1).astype(np.float32)  # (B,N)
    return pts32, sq32


def _prep_inputs(pts32, sq32):
    aux = np.empty((1, 128 + N + 1024), np.float32)
    aux[0, :128] = 1.0
    aux[0, 128:128 + N] = _tf32(np.arange(N, dtype=np.float64) % GRP)
    aux[0, 128 + N:128 + N + CHUNK] = BIG
    aux[0, 128 + N + CHUNK:] = -BIG
    idxrow = aux
    idx128 = np.broadcast_to(np.arange(CHUNK, dtype=np.float32)[None, :],
                             (128, CHUNK)).copy()
    in_maps = []
    for c in range(NCORES):
        b, h = c // 2, c % 2
        r0 = h * ROWS_PER_CORE
        packed = np.empty((KROWS, ROWS_PER_CORE + N), dtype=np.float32)
        packed[:C, :ROWS_PER_CORE] = _tf32(
            (2.0 * pts32[b, r0:r0 + ROWS_PER_CORE].astype(np.float64) * K1).astype(np.float32)).T
        packed[C, :ROWS_PER_CORE] = K1
        packed[C + 1, :ROWS_PER_CORE] = K1
        packed[C + 2, :ROWS_PER_CORE] = 1.0
        packed[C + 3, :ROWS_PER_CORE] = 1.0
        packed[:C, ROWS_PER_CORE:] = _tf32(pts32[b]).T
        u = (OFFC - sq32[b].astype(np.float64)).astype(np.float32)
        uh = _tf32(u)
        ul = _tf32((u.astype(np.float64) - uh.astype(np.float64)).astype(np.float32))
        packed[C, ROWS_PER_CORE:] = uh
        packed[C + 1, ROWS_PER_CORE:] = ul
        packed[C + 2, ROWS_PER_CORE:] = BIG
        packed[C + 3, ROWS_PER_CORE:] = -BIG
        in_maps.append({"packed": packed, "idxrow": idxrow, "idx128": idx128})
    return in_maps


def _rescore_topk(pts32, sq32, cand_b, b):
    """Exact rescoring of device candidates; returns (N, KOUT) neighbor ids."""
    a = pts32[b].astype(np.float64)          # (N, C)
    sq = sq32[b].astype(np.float64)          # (N,)
    nn = np.empty((N, KOUT), dtype=np.int32)
    BLK = 2048
    need = 2 * KOUT - 1                      # ranks 0..16
    for s0 in range(0, N, BLK):
        s1 = min(N, s0 + BLK)
        cb = cand_b[s0:s1]                   # (blk, NCAND)
        bsel = a[cb]                         # (blk, NCAND, C)
        s = 2.0 * np.einsum('nc,nkc->nk', a[s0:s1], bsel) - sq[cb]
        # order by (-score, candidate id) to mirror top_k's stable tie-break
        o = np.lexsort((cb, -s), axis=1)[:, :need]
        top = np.take_along_axis(cb, o, axis=1)
        nn[s0:s1] = top[:, 0:need:2].astype(np.int32)
    return nn


def _run(x, trace=False):
    nc = _get_nc()
    pts32, sq32 = _normalize(x)
    in_maps = _prep_inputs(pts32, sq32)
    last_err = None
    for attempt in range(4):
        try:
            res = run_bass_kernel_spmd(nc, in_maps, list(range(NCORES)), trace=trace)
            if _results_valid(res.results):
                break
            last_err = RuntimeError("device returned invalid candidate values")
        except Exception as e:
            last_err = e
        import time as _time
        _time.sleep(1.0 + attempt)
    else:
        raise last_err
    per_core = [_decode_cands(res.results, c) for c in range(NCORES)]
    nn9 = np.empty((B, N, KOUT), dtype=np.int32)
    for b in range(B):
        cand_b = np.concatenate([per_core[2 * b], per_core[2 * b + 1]], axis=0)
        nn9[b] = _rescore_topk(pts32, sq32, cand_b, b)
    center = np.broadcast_to(np.arange(N, dtype=np.int32)[None, :, None],
                             (B, N, KOUT))
    edge_index = np.stack([nn9, np.ascontiguousarray(center)], axis=0)
    return edge_index, res


def kernel(x):
    edge_index, _ = _run(x, trace=False)
    return edge_index
